# revision 27
# baseline (speedup 1.0000x reference)
"""Multi-head graph attention layer (GCN-conv QKV + per-edge attention)
on 8 Trainium2 NeuronCores.

Strategy (edge-parallel by dst-owner + node data parallel):
  - Nodes are partitioned across 8 cores (NPC = N/8 per core).  Edges are
    bucketed by the OWNER OF THEIR DST node, so every segment-sum
    (aggregate, wV, z) is core-local: no all-reduce of partials.
  - Within a core, edges are sorted by dst and grouped by 128-node groups.
    A segment-sum over a 128-edge tile is a matmul with a 0/1 selection
    matrix M[e, r] = (dstloc[e] == r) built on-chip with is_equal vs iota.
  - Per-edge gathers of source-node features use gpsimd indirect DMA
    (int32 row indices; one instruction gathers a whole group's edges:
    dest slot [p, t] receives row idx[p, t]).
  - K/V node features are exchanged between cores with one AllGather.

All floating point math happens on-device; the host only does integer
graph preprocessing (bucketing/sorting/padding of edge indices, degree
counting) and the final slice concatenation.
"""

import numpy as np

P = 128
NCORES = 8
SUB = 4  # elementwise op fusion width (tiles per DVE op)


# ----------------------------------------------------------------------------
# Host-side integer preprocessing
# ----------------------------------------------------------------------------
def _host_prep(N, E, HID, HEADS, src, dst):
    NPC = N // NCORES
    assert NPC * NCORES == N, "N must be divisible by 8"
    G = -(-NPC // P)
    NPAD = G * P
    KVR = NCORES * NPAD

    src = np.asarray(src, np.int64)
    dst = np.asarray(dst, np.int64)

    deg_out = np.bincount(src, minlength=N).astype(np.int32)
    deg_in = np.bincount(dst, minlength=N).astype(np.int32)

    kvsrc = (src // NPC) * NPAD + (src % NPC)
    owner = dst // NPC
    r_local = dst % NPC
    g_arr = r_local // P
    rloc = r_local % P

    # counts per (core, group)
    key_full = owner * G + g_arr
    cnt = np.bincount(key_full, minlength=NCORES * G).reshape(NCORES, G)
    T_g = np.maximum(-(-cnt.max(axis=0) // P), 0).astype(np.int64)
    tbase = np.zeros(G + 1, np.int64)
    tbase[1:] = np.cumsum(T_g)
    TILES = int(tbase[-1])

    per_core = []
    for c in range(NCORES):
        sel = np.flatnonzero(owner == c)
        key = g_arr[sel]
        order = np.argsort(key, kind="stable")
        se = sel[order]
        ks = key[order]
        cnts = np.bincount(ks, minlength=G)
        starts = np.concatenate([[0], np.cumsum(cnts)[:-1]])
        within = np.arange(se.size) - np.repeat(starts, cnts)
        slot = tbase[ks] * P + within

        idx32 = np.zeros((P, max(TILES, 1)), np.int32)
        idx32[slot % P, slot // P] = kvsrc[se].astype(np.int32)

        dstloc = np.full((P, max(TILES, 1)), -1.0, np.float32)
        dstloc[slot % P, slot // P] = rloc[se].astype(np.float32)

        # per-core in-degree, partition-inner layout [128, G]
        di = np.ones(NPAD, np.int32)
        di[:NPC] = deg_in[c * NPC : (c + 1) * NPC]
        deg_in_pi = di.reshape(G, P).T.copy()

        per_core.append(
            {
                "idx32": idx32,
                "dstloc": np.ascontiguousarray(dstloc),
                "deg_in_pi": deg_in_pi,
            }
        )

    # replicated out-degree, partition-inner layout [128, NCORES*G]
    do = np.ones(NCORES * NPAD, np.int32)
    for c in range(NCORES):
        do[c * NPAD : c * NPAD + NPC] = deg_out[c * NPC : (c + 1) * NPC]
    deg_out_pi = do.reshape(NCORES * G, P).T.copy()

    cfg = dict(
        N=N, E=E, HID=HID, HEADS=HEADS, HD=HID // HEADS,
        NPC=NPC, G=G, NPAD=NPAD, KVR=KVR,
        T_g=[int(x) for x in T_g],
        tbase=[int(x) for x in tbase], TILES=TILES,
    )
    return cfg, per_core, deg_out_pi


def _split_sync_waits(nc, max_waits=1, nop_waits=1):
    """The walrus build in this container rejects instructions carrying more
    than a couple of semaphore waits (DMA encodings allow just one).  Move
    excess waits onto NoOp instructions inserted just before the offender on
    the same engine."""
    import concourse.mybir as mybir

    n_split = 0
    for bb in nc.main_func.blocks:
        insts = bb.instructions
        i = 0
        while i < len(insts):
            ins = insts[i]
            si = ins.sync_info
            if si is not None and len(si.on_wait) > max_waits:
                waits = list(si.on_wait)
                keep = waits[-max_waits:]
                extra = waits[:-max_waits]
                ins.sync_info = mybir.SyncInfo(
                    on_wait=keep, on_update=list(si.on_update)
                )
                pos = i
                while extra:
                    chunk, extra = extra[:nop_waits], extra[nop_waits:]
                    nop = mybir.InstNoOp(name=f"I-wsplit{n_split}", ins=[], outs=[])
                    n_split += 1
                    nop.engine = ins.engine
                    nop.sync_info = mybir.SyncInfo(on_wait=chunk, on_update=[])
                    insts.insert(pos, nop)
                    pos += 1
                    i += 1
            i += 1
    return n_split


# ----------------------------------------------------------------------------
# Device program
# ----------------------------------------------------------------------------
def _build_program(cfg, has_bias, debug=False):
    import concourse.bass as bass
    import concourse.mybir as mybir
    import concourse.tile as tile
    from concourse.masks import make_identity

    dt = mybir.dt
    f32 = dt.float32
    Alu = mybir.AluOpType
    Act = mybir.ActivationFunctionType

    N, HID, HEADS, HD = cfg["N"], cfg["HID"], cfg["HEADS"], cfg["HD"]
    NPC, G, NPAD, KVR = cfg["NPC"], cfg["G"], cfg["NPAD"], cfg["KVR"]
    T_g, tbase, TILES = cfg["T_g"], cfg["tbase"], cfg["TILES"]
    NC8G = NCORES * G
    FT = NPC // P          # full 128-row tiles per node chunk
    REM = NPC - FT * P     # leftover rows in the last tile
    C2 = HID + HEADS       # wVz psum columns

    nc = bass.Bass()
    h_e = nc.declare_dram_parameter("h", [N, HID], f32, isOutput=False)
    wq_e = nc.declare_dram_parameter("wq", [HID, HID], f32, isOutput=False)
    wk_e = nc.declare_dram_parameter("wk", [HID, HID], f32, isOutput=False)
    wv_e = nc.declare_dram_parameter("wv", [HID, HID], f32, isOutput=False)
    dout_e = nc.declare_dram_parameter("deg_out_pi", [P, NC8G], dt.int32, isOutput=False)
    din_e = nc.declare_dram_parameter("deg_in_pi", [P, G], dt.int32, isOutput=False)
    idx_e = nc.declare_dram_parameter("idx32", [P, max(TILES, 1)], dt.int32, isOutput=False)
    dl_e = nc.declare_dram_parameter("dstloc", [P, max(TILES, 1)], f32, isOutput=False)
    if has_bias:
        b_e = nc.declare_dram_parameter("bias3", [3, HID], f32, isOutput=False)
    out_e = nc.declare_dram_parameter("out", [NPAD, HID], f32, isOutput=True)

    hw_d = nc.dram_tensor("hw_buf", [KVR, HID], f32)
    kvl_d = nc.dram_tensor("kv_local", [NPAD, 2 * HID], f32)
    kvf_d = nc.dram_tensor("kv_full", [KVR, 2 * HID], f32, addr_space="Shared")
    if debug:
        dbg_hw = nc.declare_dram_parameter("dbg_hw", [KVR, HID], f32, isOutput=True)
        dbg_kvf = nc.declare_dram_parameter(
            "dbg_kvf", [KVR, 2 * HID], f32, isOutput=True
        )
        dbg_q = nc.declare_dram_parameter("dbg_q", [P, NPAD], f32, isOutput=True)
        dbg_agg = nc.declare_dram_parameter("dbg_agg", [G * P, P], f32, isOutput=True)
        T0 = max(T_g[0], 1)
        dbg_buf = nc.declare_dram_parameter("dbg_buf", [P, T0 * HID], f32, isOutput=True)
        dbg_m4 = nc.declare_dram_parameter("dbg_m4", [P, min(SUB, T0) * P], f32, isOutput=True)

    with tile.TileContext(nc) as tc:
        with tc.tile_pool(name="const", bufs=1) as cp:
            # ---- constants & metadata ----
            idn = cp.tile([P, P], f32)
            make_identity(nc, idn[:])
            it_i = cp.tile([P, SUB * P], dt.int32)
            nc.gpsimd.iota(it_i[:], pattern=[[0, SUB], [1, P]], base=0,
                           channel_multiplier=0)
            iota4 = cp.tile([P, SUB * P], f32)
            nc.vector.tensor_copy(iota4[:], it_i[:])

            idx_sb = cp.tile([P, max(TILES, 1)], dt.int32)
            nc.sync.dma_start(out=idx_sb[:], in_=idx_e[:])
            dl_sb = cp.tile([P, max(TILES, 1)], f32)
            nc.sync.dma_start(out=dl_sb[:], in_=dl_e[:])

            wq_sb = cp.tile([P, HID], f32)
            wk_sb = cp.tile([P, HID], f32)
            wv_sb = cp.tile([P, HID], f32)
            nc.sync.dma_start(out=wq_sb[:], in_=wq_e[:])
            nc.sync.dma_start(out=wk_sb[:], in_=wk_e[:])
            nc.sync.dma_start(out=wv_sb[:], in_=wv_e[:])

            # norm_out = rsqrt(max(deg_out, 1)) over all cores' nodes
            no_i = cp.tile([P, NC8G], dt.int32)
            nc.sync.dma_start(out=no_i[:], in_=dout_e[:])
            no_f = cp.tile([P, NC8G], f32)
            nc.vector.tensor_copy(no_f[:], no_i[:])
            nc.vector.tensor_scalar_max(no_f[:], no_f[:], 1.0)
            nc.vector.reciprocal(no_f[:], no_f[:])
            nc.scalar.sqrt(no_f[:], no_f[:])

            ni_i = cp.tile([P, G], dt.int32)
            nc.sync.dma_start(out=ni_i[:], in_=din_e[:])
            ni_f = cp.tile([P, G], f32)
            nc.vector.tensor_copy(ni_f[:], ni_i[:])
            nc.vector.tensor_scalar_max(ni_f[:], ni_f[:], 1.0)
            nc.vector.reciprocal(ni_f[:], ni_f[:])
            nc.scalar.sqrt(ni_f[:], ni_f[:])
            niq = cp.tile([P, G], f32)  # norm_in scaled by 1/sqrt(HD) for Q
            nc.vector.tensor_scalar_mul(niq[:], ni_f[:], 1.0 / float(np.sqrt(HD)))

            if has_bias:
                ones1 = cp.tile([1, P], f32)
                nc.vector.memset(ones1[:], 1.0)
                b_reps = []
                with tc.tile_pool(name="bps", bufs=1, space="PSUM") as bps:
                    for i in range(3):
                        brow = cp.tile([1, HID], f32, tag=f"brow{i}")
                        nc.sync.dma_start(out=brow[:], in_=b_e[i : i + 1, :])
                        ps = bps.tile([P, HID], f32, tag="brep")
                        nc.tensor.matmul(out=ps[:], lhsT=ones1[:],
                                         rhs=brow[:], start=True, stop=True)
                        br = cp.tile([P, HID], f32, tag=f"brep{i}")
                        nc.scalar.copy(br[:], ps[:])
                        b_reps.append(br)

            Q_all = cp.tile([P, NPAD], f32)

            # ---- prepass: hw = h * norm_out, stored in kv-row numbering ----
            with tc.tile_pool(name="prep", bufs=2) as pp:
                for c in range(NCORES):
                    if FT:
                        hs = pp.tile([P, FT * HID], f32, tag="hslab")
                        hs3 = hs[:].rearrange("p (t d) -> p t d", d=HID)
                        nc.sync.dma_start(
                            out=hs3,
                            in_=h_e[c * NPC : c * NPC + FT * P, :].rearrange(
                                "(t p) d -> p t d", p=P
                            ),
                        )
                        nc.vector.tensor_tensor(
                            out=hs3,
                            in0=hs3,
                            in1=no_f[:, c * G : c * G + FT].to_broadcast([P, FT, HID]),
                            op=Alu.mult,
                        )
                        nc.sync.dma_start(
                            out=hw_d[c * NPAD : c * NPAD + FT * P, :].rearrange(
                                "(t p) d -> p t d", p=P
                            ),
                            in_=hs3,
                        )
                    if REM:
                        ht = pp.tile([P, HID], f32, tag="hrem")
                        nc.sync.dma_start(
                            out=ht[:REM],
                            in_=h_e[c * NPC + FT * P : (c + 1) * NPC, :],
                        )
                        nc.vector.tensor_tensor(
                            out=ht[:REM],
                            in0=ht[:REM],
                            in1=no_f[:REM, c * G + FT : c * G + FT + 1].to_broadcast(
                                [REM, HID]
                            ),
                            op=Alu.mult,
                        )
                        nc.sync.dma_start(
                            out=hw_d[c * NPAD + FT * P : c * NPAD + FT * P + REM, :],
                            in_=ht[:REM],
                        )

            # ---- phase B: aggregate + QKV projection ----
            with (
                tc.tile_pool(name="pb", bufs=2) as pb,
                tc.tile_pool(name="psB", bufs=2, space="PSUM") as psB,
            ):
                for g in range(G):
                    TT = T_g[g]
                    colbase = tbase[g]
                    aggT_s = pb.tile([P, P], f32, tag="aggTs")
                    if TT == 0:
                        nc.vector.memset(aggT_s[:], 0.0)
                    else:
                        aggT_ps = psB.tile([P, P], f32, tag="aggT")
                        buf = pb.tile([P, TT * HID], f32, tag="gbuf")
                        for t in range(TT):
                            nc.gpsimd.indirect_dma_start(
                                out=buf[:, t * HID : (t + 1) * HID],
                                out_offset=None,
                                in_=hw_d[:, :],
                                in_offset=bass.IndirectOffsetOnAxis(
                                    ap=idx_sb[:, colbase + t : colbase + t + 1],
                                    axis=0,
                                ),
                            )
                        mm_i = 0
                        for t0 in range(0, TT, SUB):
                            w = min(SUB, TT - t0)
                            m4 = pb.tile([P, w * P], f32, tag="m4")
                            nc.vector.tensor_tensor(
                                out=m4[:].rearrange("p (t r) -> p t r", r=P),
                                in0=dl_sb[
                                    :, colbase + t0 : colbase + t0 + w
                                ].to_broadcast([P, w, P]),
                                in1=iota4[:, : w * P].rearrange(
                                    "p (t r) -> p t r", r=P
                                ),
                                op=Alu.is_equal,
                            )
                            if debug and g == 0 and t0 == 0:
                                nc.sync.dma_start(out=dbg_m4[:], in_=m4[:])
                            for k in range(w):
                                t = t0 + k
                                nc.tensor.matmul(
                                    out=aggT_ps[:],
                                    lhsT=buf[:, t * HID : (t + 1) * HID],
                                    rhs=m4[:, k * P : (k + 1) * P],
                                    start=(mm_i == 0),
                                    stop=(mm_i == TT - 1),
                                )
                                mm_i += 1
                        if debug and g == 0:
                            nc.sync.dma_start(out=dbg_buf[:], in_=buf[:])
                        nc.scalar.copy(aggT_s[:], aggT_ps[:])
                    if debug:
                        nc.sync.dma_start(
                            out=dbg_agg[g * P : (g + 1) * P, :], in_=aggT_s[:]
                        )

                    kv_sb = pb.tile([P, 2 * HID], f32, tag="kvsb")
                    plans = (
                        (wq_sb, Q_all[:, g * P : (g + 1) * P], niq, 0),
                        (wk_sb, kv_sb[:, :HID], ni_f, 1),
                        (wv_sb, kv_sb[:, HID:], ni_f, 2),
                    )
                    for w_sb, outap, scol, bi in plans:
                        ps = psB.tile([P, HID], f32, tag="qkv")
                        nc.tensor.matmul(
                            out=ps[:], lhsT=aggT_s[:], rhs=w_sb[:],
                            start=True, stop=True,
                        )
                        if has_bias:
                            tmp = pb.tile([P, HID], f32, tag="btmp")
                            nc.scalar.activation(
                                out=tmp[:], in_=ps[:], func=Act.Copy,
                                scale=ni_f[:, g : g + 1],
                            )
                            nc.vector.tensor_tensor(
                                out=tmp[:], in0=tmp[:], in1=b_reps[bi][:], op=Alu.add
                            )
                            nc.vector.tensor_scalar(
                                out=outap, in0=tmp[:],
                                scalar1=0.0,
                                scalar2=(1.0 / float(np.sqrt(HD)) if bi == 0 else 1.0),
                                op0=Alu.max, op1=Alu.mult,
                            )
                        else:
                            nc.scalar.activation(
                                out=outap, in_=ps[:], func=Act.Relu,
                                scale=scol[:, g : g + 1],
                            )
                    nc.sync.dma_start(
                        out=kvl_d[g * P : (g + 1) * P, :], in_=kv_sb[:]
                    )

            # ---- K/V exchange ----
            nc.gpsimd.collective_compute(
                "AllGather",
                Alu.bypass,
                replica_groups=[list(range(NCORES))],
                ins=[kvl_d[:]],
                outs=[kvf_d[:]],
            )
            if debug:
                nc.sync.dma_start(out=dbg_hw[:], in_=hw_d[:])
                nc.sync.dma_start(out=dbg_kvf[:], in_=kvf_d[:])
                nc.sync.dma_start(out=dbg_q[:], in_=Q_all[:])

            # ---- phase C: per-edge attention ----
            with (
                tc.tile_pool(name="pc", bufs=2) as pc,
                tc.tile_pool(name="psC", bufs=2, space="PSUM") as psC,
            ):
                for g in range(G):
                    TT = T_g[g]
                    if TT == 0:
                        ob = pc.tile([P, HID], f32, tag="outsb")
                        nc.vector.memset(ob[:], 0.0)
                        nc.sync.dma_start(
                            out=out_e[g * P : (g + 1) * P, :], in_=ob[:]
                        )
                        continue
                    wvz_ps = psC.tile([P, C2], f32, tag="wvz")
                    mm_i = 0
                    for T, colbase in ((TT, tbase[g]),):
                        buf = pc.tile([P, T * 2 * HID], f32, tag="kvbuf")
                        b3 = buf[:].rearrange("p (t x) -> p t x", x=2 * HID)
                        for t in range(T):
                            nc.gpsimd.indirect_dma_start(
                                out=buf[:, t * 2 * HID : (t + 1) * 2 * HID],
                                out_offset=None,
                                in_=kvf_d[:, :],
                                in_offset=bass.IndirectOffsetOnAxis(
                                    ap=idx_sb[:, colbase + t : colbase + t + 1],
                                    axis=0,
                                ),
                            )
                        for t0 in range(0, T, SUB):
                            w = min(SUB, T - t0)
                            m4 = pc.tile([P, w * P], f32, tag="m4c")
                            nc.vector.tensor_tensor(
                                out=m4[:].rearrange("p (t r) -> p t r", r=P),
                                in0=dl_sb[
                                    :, colbase + t0 : colbase + t0 + w
                                ].to_broadcast([P, w, P]),
                                in1=iota4[:, : w * P].rearrange(
                                    "p (t r) -> p t r", r=P
                                ),
                                op=Alu.is_equal,
                            )
                            mt_ps = psC.tile([P, w * P], f32, tag="mt4")
                            for k in range(w):
                                nc.tensor.transpose(
                                    out=mt_ps[:, k * P : (k + 1) * P],
                                    in_=m4[:, k * P : (k + 1) * P],
                                    identity=idn[:],
                                )
                            mt_s = pc.tile([P, w * P], f32, tag="mt4s")
                            nc.scalar.copy(mt_s[:], mt_ps[:])
                            qe_ps = psC.tile([P, w * P], f32, tag="qe4")
                            for k in range(w):
                                nc.tensor.matmul(
                                    out=qe_ps[:, k * P : (k + 1) * P],
                                    lhsT=mt_s[:, k * P : (k + 1) * P],
                                    rhs=Q_all[:, g * P : (g + 1) * P],
                                    start=True,
                                    stop=True,
                                )
                            kq = pc.tile([P, w * P], f32, tag="kq")
                            nc.vector.tensor_tensor(
                                out=kq[:].rearrange("p (t d) -> p t d", d=HID),
                                in0=b3[:, t0 : t0 + w, :HID],
                                in1=qe_ps[:].rearrange("p (t d) -> p t d", d=HID)[
                                    :, :w, :
                                ],
                                op=Alu.mult,
                            )
                            s4 = pc.tile([P, w * HEADS], f32, tag="s4")
                            nc.vector.tensor_reduce(
                                out=s4[:],
                                in_=kq[:, : w * P].rearrange(
                                    "p (a x) -> p a x", x=HD
                                ),
                                axis=mybir.AxisListType.X,
                                op=Alu.add,
                            )
                            nc.vector.tensor_scalar(
                                out=s4[:], in0=s4[:],
                                scalar1=10.0, scalar2=-10.0,
                                op0=Alu.min, op1=Alu.max,
                            )
                            rhs4 = pc.tile([P, w * C2], f32, tag="rhs4")
                            r3 = rhs4[:].rearrange("p (t c) -> p t c", c=C2)
                            nc.scalar.activation(
                                out=r3[:, :, HID:],
                                in_=s4[:].rearrange("p (t a) -> p t a", a=HEADS),
                                func=Act.Exp,
                            )
                            nc.vector.tensor_tensor(
                                out=r3[:, :, :HID].rearrange(
                                    "p t (a x) -> p t a x", x=HD
                                ),
                                in0=b3[:, t0 : t0 + w, HID:].rearrange(
                                    "p t (a x) -> p t a x", x=HD
                                ),
                                in1=r3[:, :, HID:].to_broadcast([P, w, HEADS, HD]),
                                op=Alu.mult,
                            )
                            for k in range(w):
                                nc.tensor.matmul(
                                    out=wvz_ps[:],
                                    lhsT=m4[:, k * P : (k + 1) * P],
                                    rhs=rhs4[:, k * C2 : (k + 1) * C2],
                                    start=(mm_i == 0),
                                    stop=(mm_i == TT - 1),
                                )
                                mm_i += 1
                    zt = pc.tile([P, HEADS], f32, tag="zt")
                    nc.vector.tensor_scalar_add(zt[:], wvz_ps[:, HID:], 1e-6)
                    rz = pc.tile([P, HEADS], f32, tag="rz")
                    nc.vector.reciprocal(rz[:], zt[:])
                    ob = pc.tile([P, HID], f32, tag="outsb")
                    nc.vector.tensor_tensor(
                        out=ob[:].rearrange("p (a x) -> p a x", x=HD),
                        in0=wvz_ps[:, :HID].rearrange("p (a x) -> p a x", x=HD),
                        in1=rz[:].to_broadcast([P, HEADS, HD]),
                        op=Alu.mult,
                    )
                    nc.sync.dma_start(
                        out=out_e[g * P : (g + 1) * P, :], in_=ob[:]
                    )
    _split_sync_waits(nc)
    return nc


# ----------------------------------------------------------------------------
# Entry point
# ----------------------------------------------------------------------------
def _run(h, W_Q, b_Q, W_K, b_K, W_V, b_V, src, dst, trace=False, tmpdir=None,
         debug=False):
    from concourse.bass_utils import run_bass_kernel_spmd

    h = np.ascontiguousarray(h, np.float32)
    N, HID = h.shape
    HEADS = 8
    E = len(src)

    cfg, per_core, deg_out_pi = _host_prep(N, E, HID, HEADS, src, dst)
    has_bias = bool(
        np.any(np.asarray(b_Q)) or np.any(np.asarray(b_K)) or np.any(np.asarray(b_V))
    )
    nc = _build_program(cfg, has_bias, debug=debug)

    shared = {
        "h": h,
        "wq": np.ascontiguousarray(W_Q, np.float32),
        "wk": np.ascontiguousarray(W_K, np.float32),
        "wv": np.ascontiguousarray(W_V, np.float32),
        "deg_out_pi": deg_out_pi,
    }
    if has_bias:
        shared["bias3"] = np.ascontiguousarray(
            np.stack([b_Q, b_K, b_V]).astype(np.float32)
        )
    in_maps = [dict(shared, **pc) for pc in per_core]
    res = run_bass_kernel_spmd(
        nc, in_maps, list(range(NCORES)), trace=trace, tmpdir=tmpdir
    )
    NPC = cfg["NPC"]
    out = np.concatenate(
        [res.results[c]["out"][:NPC] for c in range(NCORES)], axis=0
    )
    return out, res


def kernel(**inputs):
    out, _ = _run(
        inputs["h"],
        inputs["W_Q"], inputs["b_Q"],
        inputs["W_K"], inputs["b_K"],
        inputs["W_V"], inputs["b_V"],
        np.asarray(inputs["src"]), np.asarray(inputs["dst"]),
    )
    return out.astype(np.float32)


# revision 28
# speedup vs baseline: 1.1224x; 1.1224x over previous
"""Multi-head graph attention layer (GCN-conv QKV + per-edge attention)
on 8 Trainium2 NeuronCores.

Strategy (edge-parallel by dst-owner + node data parallel):
  - Nodes are partitioned across 8 cores (NPC = N/8 per core).  Edges are
    bucketed by the OWNER OF THEIR DST node, so every segment-sum
    (aggregate, wV, z) is core-local: no all-reduce of partials.
  - Within a core, edges are sorted by dst and grouped by 128-node groups.
    A segment-sum over a 128-edge tile is a matmul with a 0/1 selection
    matrix M[e, r] = (dstloc[e] == r) built on-chip with is_equal vs iota.
  - Per-edge gathers of source-node features use gpsimd indirect DMA
    (int32 row indices; one instruction gathers a whole group's edges:
    dest slot [p, t] receives row idx[p, t]).
  - K/V node features are exchanged between cores with one AllGather.

All floating point math happens on-device; the host only does integer
graph preprocessing (bucketing/sorting/padding of edge indices, degree
counting) and the final slice concatenation.
"""

import numpy as np

P = 128
NCORES = 8
SUB = 4  # elementwise op fusion width (tiles per DVE op)


# ----------------------------------------------------------------------------
# Host-side integer preprocessing
# ----------------------------------------------------------------------------
def _host_prep(N, E, HID, HEADS, src, dst):
    NPC = N // NCORES
    assert NPC * NCORES == N, "N must be divisible by 8"
    G = -(-NPC // P)
    NPAD = G * P
    KVR = NCORES * NPAD

    src = np.asarray(src, np.int64)
    dst = np.asarray(dst, np.int64)

    deg_out = np.bincount(src, minlength=N).astype(np.int32)
    deg_in = np.bincount(dst, minlength=N).astype(np.int32)

    kvsrc = (src // NPC) * NPAD + (src % NPC)
    owner = dst // NPC
    r_local = dst % NPC
    g_arr = r_local // P
    rloc = r_local % P

    # counts per (core, group)
    key_full = owner * G + g_arr
    cnt = np.bincount(key_full, minlength=NCORES * G).reshape(NCORES, G)
    T_g = np.maximum(-(-cnt.max(axis=0) // P), 0).astype(np.int64)
    tbase = np.zeros(G + 1, np.int64)
    tbase[1:] = np.cumsum(T_g)
    TILES = int(tbase[-1])

    per_core = []
    for c in range(NCORES):
        sel = np.flatnonzero(owner == c)
        key = g_arr[sel]
        order = np.argsort(key, kind="stable")
        se = sel[order]
        ks = key[order]
        cnts = np.bincount(ks, minlength=G)
        starts = np.concatenate([[0], np.cumsum(cnts)[:-1]])
        within = np.arange(se.size) - np.repeat(starts, cnts)
        slot = tbase[ks] * P + within

        idx32 = np.zeros((P, max(TILES, 1)), np.int32)
        idx32[slot % P, slot // P] = kvsrc[se].astype(np.int32)

        dstloc = np.full((P, max(TILES, 1)), -1.0, np.float32)
        dstloc[slot % P, slot // P] = rloc[se].astype(np.float32)

        # per-core in-degree, partition-inner layout [128, G]
        di = np.ones(NPAD, np.int32)
        di[:NPC] = deg_in[c * NPC : (c + 1) * NPC]
        deg_in_pi = di.reshape(G, P).T.copy()

        per_core.append(
            {
                "idx32": idx32,
                "dstloc": np.ascontiguousarray(dstloc),
                "deg_in_pi": deg_in_pi,
            }
        )

    # replicated out-degree, partition-inner layout [128, NCORES*G]
    do = np.ones(NCORES * NPAD, np.int32)
    for c in range(NCORES):
        do[c * NPAD : c * NPAD + NPC] = deg_out[c * NPC : (c + 1) * NPC]
    deg_out_pi = do.reshape(NCORES * G, P).T.copy()

    cfg = dict(
        N=N, E=E, HID=HID, HEADS=HEADS, HD=HID // HEADS,
        NPC=NPC, G=G, NPAD=NPAD, KVR=KVR,
        T_g=[int(x) for x in T_g],
        tbase=[int(x) for x in tbase], TILES=TILES,
    )
    return cfg, per_core, deg_out_pi


def _split_sync_waits(nc, max_waits=1, nop_waits=1):
    """The walrus build in this container rejects instructions carrying more
    than a couple of semaphore waits (DMA encodings allow just one).  Move
    excess waits onto NoOp instructions inserted just before the offender on
    the same engine."""
    import concourse.mybir as mybir

    n_split = 0
    for bb in nc.main_func.blocks:
        insts = bb.instructions
        i = 0
        while i < len(insts):
            ins = insts[i]
            si = ins.sync_info
            if si is not None and len(si.on_wait) > max_waits:
                waits = list(si.on_wait)
                keep = waits[-max_waits:]
                extra = waits[:-max_waits]
                ins.sync_info = mybir.SyncInfo(
                    on_wait=keep, on_update=list(si.on_update)
                )
                pos = i
                while extra:
                    chunk, extra = extra[:nop_waits], extra[nop_waits:]
                    nop = mybir.InstNoOp(name=f"I-wsplit{n_split}", ins=[], outs=[])
                    n_split += 1
                    nop.engine = ins.engine
                    nop.sync_info = mybir.SyncInfo(on_wait=chunk, on_update=[])
                    insts.insert(pos, nop)
                    pos += 1
                    i += 1
            i += 1
    return n_split


# ----------------------------------------------------------------------------
# Device program
# ----------------------------------------------------------------------------
def _build_program(cfg, has_bias, debug=False):
    import concourse.bass as bass
    import concourse.mybir as mybir
    import concourse.tile as tile
    from concourse.masks import make_identity

    dt = mybir.dt
    f32 = dt.float32
    Alu = mybir.AluOpType
    Act = mybir.ActivationFunctionType

    N, HID, HEADS, HD = cfg["N"], cfg["HID"], cfg["HEADS"], cfg["HD"]
    NPC, G, NPAD, KVR = cfg["NPC"], cfg["G"], cfg["NPAD"], cfg["KVR"]
    T_g, tbase, TILES = cfg["T_g"], cfg["tbase"], cfg["TILES"]
    NC8G = NCORES * G
    FT = NPC // P          # full 128-row tiles per node chunk
    REM = NPC - FT * P     # leftover rows in the last tile
    C2 = HID + HEADS       # wVz psum columns

    nc = bass.Bass()
    h_e = nc.declare_dram_parameter("h", [N, HID], f32, isOutput=False)
    wq_e = nc.declare_dram_parameter("wq", [HID, HID], f32, isOutput=False)
    wk_e = nc.declare_dram_parameter("wk", [HID, HID], f32, isOutput=False)
    wv_e = nc.declare_dram_parameter("wv", [HID, HID], f32, isOutput=False)
    dout_e = nc.declare_dram_parameter("deg_out_pi", [P, NC8G], dt.int32, isOutput=False)
    din_e = nc.declare_dram_parameter("deg_in_pi", [P, G], dt.int32, isOutput=False)
    idx_e = nc.declare_dram_parameter("idx32", [P, max(TILES, 1)], dt.int32, isOutput=False)
    dl_e = nc.declare_dram_parameter("dstloc", [P, max(TILES, 1)], f32, isOutput=False)
    if has_bias:
        b_e = nc.declare_dram_parameter("bias3", [3, HID], f32, isOutput=False)
    out_e = nc.declare_dram_parameter("out", [NPAD, HID], f32, isOutput=True)

    bf16 = dt.bfloat16
    hw_d = nc.dram_tensor("hw_buf", [KVR, HID], bf16)
    kvlk_d = nc.dram_tensor("kv_local_k", [NPAD, HID], bf16)
    kvlv_d = nc.dram_tensor("kv_local_v", [NPAD, HID], bf16)
    kvfk_d = nc.dram_tensor("kv_full_k", [KVR, HID], bf16, addr_space="Shared")
    kvfv_d = nc.dram_tensor("kv_full_v", [KVR, HID], bf16, addr_space="Shared")
    if debug:
        dbg_hw = nc.declare_dram_parameter("dbg_hw", [KVR, HID], dt.bfloat16, isOutput=True)
        dbg_kvf = nc.declare_dram_parameter(
            "dbg_kvf", [KVR, 2 * HID], dt.bfloat16, isOutput=True
        )
        dbg_q = nc.declare_dram_parameter("dbg_q", [P, NPAD], f32, isOutput=True)
        dbg_agg = nc.declare_dram_parameter("dbg_agg", [G * P, P], f32, isOutput=True)
        T0 = max(T_g[0], 1)
        dbg_buf = nc.declare_dram_parameter("dbg_buf", [P, T0 * HID], f32, isOutput=True)
        dbg_m4 = nc.declare_dram_parameter("dbg_m4", [P, min(SUB, T0) * P], f32, isOutput=True)

    with tile.TileContext(nc) as tc:
        with tc.tile_pool(name="const", bufs=1) as cp:
            # ---- constants & metadata ----
            idn = cp.tile([P, P], f32)
            make_identity(nc, idn[:])
            it_i = cp.tile([P, SUB * P], dt.int32)
            nc.gpsimd.iota(it_i[:], pattern=[[0, SUB], [1, P]], base=0,
                           channel_multiplier=0)
            iota4 = cp.tile([P, SUB * P], f32)
            nc.vector.tensor_copy(iota4[:], it_i[:])

            idx_sb = cp.tile([P, max(TILES, 1)], dt.int32)
            nc.sync.dma_start(out=idx_sb[:], in_=idx_e[:])
            dl_sb = cp.tile([P, max(TILES, 1)], f32)
            nc.sync.dma_start(out=dl_sb[:], in_=dl_e[:])

            wq_sb = cp.tile([P, HID], f32)
            wk_sb = cp.tile([P, HID], f32)
            wv_sb = cp.tile([P, HID], f32)
            nc.sync.dma_start(out=wq_sb[:], in_=wq_e[:])
            nc.sync.dma_start(out=wk_sb[:], in_=wk_e[:])
            nc.sync.dma_start(out=wv_sb[:], in_=wv_e[:])

            # norm_out = rsqrt(max(deg_out, 1)) over all cores' nodes
            no_i = cp.tile([P, NC8G], dt.int32)
            nc.sync.dma_start(out=no_i[:], in_=dout_e[:])
            no_f = cp.tile([P, NC8G], f32)
            nc.vector.tensor_copy(no_f[:], no_i[:])
            nc.vector.tensor_scalar_max(no_f[:], no_f[:], 1.0)
            nc.vector.reciprocal(no_f[:], no_f[:])
            nc.scalar.sqrt(no_f[:], no_f[:])

            ni_i = cp.tile([P, G], dt.int32)
            nc.sync.dma_start(out=ni_i[:], in_=din_e[:])
            ni_f = cp.tile([P, G], f32)
            nc.vector.tensor_copy(ni_f[:], ni_i[:])
            nc.vector.tensor_scalar_max(ni_f[:], ni_f[:], 1.0)
            nc.vector.reciprocal(ni_f[:], ni_f[:])
            nc.scalar.sqrt(ni_f[:], ni_f[:])
            niq = cp.tile([P, G], f32)  # norm_in scaled by 1/sqrt(HD) for Q
            nc.vector.tensor_scalar_mul(niq[:], ni_f[:], 1.0 / float(np.sqrt(HD)))

            if has_bias:
                ones1 = cp.tile([1, P], f32)
                nc.vector.memset(ones1[:], 1.0)
                b_reps = []
                with tc.tile_pool(name="bps", bufs=1, space="PSUM") as bps:
                    for i in range(3):
                        brow = cp.tile([1, HID], f32, tag=f"brow{i}")
                        nc.sync.dma_start(out=brow[:], in_=b_e[i : i + 1, :])
                        ps = bps.tile([P, HID], f32, tag="brep")
                        nc.tensor.matmul(out=ps[:], lhsT=ones1[:],
                                         rhs=brow[:], start=True, stop=True)
                        br = cp.tile([P, HID], f32, tag=f"brep{i}")
                        nc.scalar.copy(br[:], ps[:])
                        b_reps.append(br)

            Q_all = cp.tile([P, NPAD], f32)

            # ---- prepass: hw = h * norm_out, stored in kv-row numbering ----
            with tc.tile_pool(name="prep", bufs=2) as pp:
                for c in range(NCORES):
                    if FT:
                        hs = pp.tile([P, FT * HID], f32, tag="hslab")
                        hs3 = hs[:].rearrange("p (t d) -> p t d", d=HID)
                        nc.sync.dma_start(
                            out=hs3,
                            in_=h_e[c * NPC : c * NPC + FT * P, :].rearrange(
                                "(t p) d -> p t d", p=P
                            ),
                        )
                        hb = pp.tile([P, FT * HID], dt.bfloat16, tag="hslab_b")
                        hb3 = hb[:].rearrange("p (t d) -> p t d", d=HID)
                        nc.vector.tensor_tensor(
                            out=hb3,
                            in0=hs3,
                            in1=no_f[:, c * G : c * G + FT].to_broadcast([P, FT, HID]),
                            op=Alu.mult,
                        )
                        nc.sync.dma_start(
                            out=hw_d[c * NPAD : c * NPAD + FT * P, :].rearrange(
                                "(t p) d -> p t d", p=P
                            ),
                            in_=hb3,
                        )
                    if REM:
                        ht = pp.tile([P, HID], f32, tag="hrem")
                        nc.sync.dma_start(
                            out=ht[:REM],
                            in_=h_e[c * NPC + FT * P : (c + 1) * NPC, :],
                        )
                        hbr = pp.tile([P, HID], dt.bfloat16, tag="hrem_b")
                        nc.vector.tensor_tensor(
                            out=hbr[:REM],
                            in0=ht[:REM],
                            in1=no_f[:REM, c * G + FT : c * G + FT + 1].to_broadcast(
                                [REM, HID]
                            ),
                            op=Alu.mult,
                        )
                        nc.sync.dma_start(
                            out=hw_d[c * NPAD + FT * P : c * NPAD + FT * P + REM, :],
                            in_=hbr[:REM],
                        )

            # ---- phase B: aggregate + QKV projection ----
            with (
                tc.tile_pool(name="pb", bufs=2) as pb,
                tc.tile_pool(name="psB", bufs=2, space="PSUM") as psB,
            ):
                for g in range(G):
                    TT = T_g[g]
                    colbase = tbase[g]
                    aggT_s = pb.tile([P, P], f32, tag="aggTs")
                    if TT == 0:
                        nc.vector.memset(aggT_s[:], 0.0)
                    else:
                        aggT_ps = psB.tile([P, P], f32, tag="aggT")
                        buf = pb.tile([P, TT * HID], dt.bfloat16, tag="gbuf")
                        for t in range(TT):
                            nc.gpsimd.indirect_dma_start(
                                out=buf[:, t * HID : (t + 1) * HID],
                                out_offset=None,
                                in_=hw_d[:, :],
                                in_offset=bass.IndirectOffsetOnAxis(
                                    ap=idx_sb[:, colbase + t : colbase + t + 1],
                                    axis=0,
                                ),
                            )
                        mm_i = 0
                        for t0 in range(0, TT, SUB):
                            w = min(SUB, TT - t0)
                            m4 = pb.tile([P, w * P], dt.bfloat16, tag="m4")
                            nc.vector.tensor_tensor(
                                out=m4[:].rearrange("p (t r) -> p t r", r=P),
                                in0=dl_sb[
                                    :, colbase + t0 : colbase + t0 + w
                                ].to_broadcast([P, w, P]),
                                in1=iota4[:, : w * P].rearrange(
                                    "p (t r) -> p t r", r=P
                                ),
                                op=Alu.is_equal,
                            )
                            if debug and g == 0 and t0 == 0:
                                nc.sync.dma_start(out=dbg_m4[:], in_=m4[:])
                            for k in range(w):
                                t = t0 + k
                                nc.tensor.matmul(
                                    out=aggT_ps[:],
                                    lhsT=buf[:, t * HID : (t + 1) * HID],
                                    rhs=m4[:, k * P : (k + 1) * P],
                                    start=(mm_i == 0),
                                    stop=(mm_i == TT - 1),
                                )
                                mm_i += 1
                        if debug and g == 0:
                            nc.sync.dma_start(out=dbg_buf[:], in_=buf[:])
                        nc.scalar.copy(aggT_s[:], aggT_ps[:])
                    if debug:
                        nc.sync.dma_start(
                            out=dbg_agg[g * P : (g + 1) * P, :], in_=aggT_s[:]
                        )

                    kv_sb = pb.tile([P, 2 * HID], dt.bfloat16, tag="kvsb")
                    plans = (
                        (wq_sb, Q_all[:, g * P : (g + 1) * P], niq, 0),
                        (wk_sb, kv_sb[:, :HID], ni_f, 1),
                        (wv_sb, kv_sb[:, HID:], ni_f, 2),
                    )
                    for w_sb, outap, scol, bi in plans:
                        ps = psB.tile([P, HID], f32, tag="qkv")
                        nc.tensor.matmul(
                            out=ps[:], lhsT=aggT_s[:], rhs=w_sb[:],
                            start=True, stop=True,
                        )
                        if has_bias:
                            tmp = pb.tile([P, HID], f32, tag="btmp")
                            nc.scalar.activation(
                                out=tmp[:], in_=ps[:], func=Act.Copy,
                                scale=ni_f[:, g : g + 1],
                            )
                            nc.vector.tensor_tensor(
                                out=tmp[:], in0=tmp[:], in1=b_reps[bi][:], op=Alu.add
                            )
                            nc.vector.tensor_scalar(
                                out=outap, in0=tmp[:],
                                scalar1=0.0,
                                scalar2=(1.0 / float(np.sqrt(HD)) if bi == 0 else 1.0),
                                op0=Alu.max, op1=Alu.mult,
                            )
                        else:
                            nc.scalar.activation(
                                out=outap, in_=ps[:], func=Act.Relu,
                                scale=scol[:, g : g + 1],
                            )
                    nc.sync.dma_start(
                        out=kvlk_d[g * P : (g + 1) * P, :], in_=kv_sb[:, :HID]
                    )
                    nc.sync.dma_start(
                        out=kvlv_d[g * P : (g + 1) * P, :], in_=kv_sb[:, HID:]
                    )

            # ---- K/V exchange ----
            nc.gpsimd.collective_compute(
                "AllGather",
                Alu.bypass,
                replica_groups=[list(range(NCORES))],
                ins=[kvlk_d[:]],
                outs=[kvfk_d[:]],
            )
            nc.gpsimd.collective_compute(
                "AllGather",
                Alu.bypass,
                replica_groups=[list(range(NCORES))],
                ins=[kvlv_d[:]],
                outs=[kvfv_d[:]],
            )
            if debug:
                nc.sync.dma_start(out=dbg_hw[:], in_=hw_d[:])
                nc.sync.dma_start(out=dbg_kvf[:, :HID], in_=kvfk_d[:])
                nc.sync.dma_start(out=dbg_kvf[:, HID:], in_=kvfv_d[:])
                nc.sync.dma_start(out=dbg_q[:], in_=Q_all[:])

            # ---- phase C: per-edge attention ----
            with (
                tc.tile_pool(name="pc", bufs=2) as pc,
                tc.tile_pool(name="psC", bufs=2, space="PSUM") as psC,
            ):
                for g in range(G):
                    TT = T_g[g]
                    if TT == 0:
                        ob = pc.tile([P, HID], f32, tag="outsb")
                        nc.vector.memset(ob[:], 0.0)
                        nc.sync.dma_start(
                            out=out_e[g * P : (g + 1) * P, :], in_=ob[:]
                        )
                        continue
                    wvz_ps = psC.tile([P, C2], f32, tag="wvz")
                    mm_i = 0
                    for T, colbase in ((TT, tbase[g]),):
                        bufk = pc.tile([P, T * HID], dt.bfloat16, tag="kbuf")
                        bufv = pc.tile([P, T * HID], dt.bfloat16, tag="vbuf")
                        bk3 = bufk[:].rearrange("p (t x) -> p t x", x=HID)
                        bv3 = bufv[:].rearrange("p (t x) -> p t x", x=HID)
                        for t in range(T):
                            nc.gpsimd.indirect_dma_start(
                                out=bufk[:, t * HID : (t + 1) * HID],
                                out_offset=None,
                                in_=kvfk_d[:, :],
                                in_offset=bass.IndirectOffsetOnAxis(
                                    ap=idx_sb[:, colbase + t : colbase + t + 1],
                                    axis=0,
                                ),
                            )
                            nc.gpsimd.indirect_dma_start(
                                out=bufv[:, t * HID : (t + 1) * HID],
                                out_offset=None,
                                in_=kvfv_d[:, :],
                                in_offset=bass.IndirectOffsetOnAxis(
                                    ap=idx_sb[:, colbase + t : colbase + t + 1],
                                    axis=0,
                                ),
                            )
                        for t0 in range(0, T, SUB):
                            w = min(SUB, T - t0)
                            m4 = pc.tile([P, w * P], f32, tag="m4c")
                            nc.vector.tensor_tensor(
                                out=m4[:].rearrange("p (t r) -> p t r", r=P),
                                in0=dl_sb[
                                    :, colbase + t0 : colbase + t0 + w
                                ].to_broadcast([P, w, P]),
                                in1=iota4[:, : w * P].rearrange(
                                    "p (t r) -> p t r", r=P
                                ),
                                op=Alu.is_equal,
                            )
                            mt_ps = psC.tile([P, w * P], f32, tag="mt4")
                            for k in range(w):
                                nc.tensor.transpose(
                                    out=mt_ps[:, k * P : (k + 1) * P],
                                    in_=m4[:, k * P : (k + 1) * P],
                                    identity=idn[:],
                                )
                            mt_s = pc.tile([P, w * P], f32, tag="mt4s")
                            nc.scalar.copy(mt_s[:], mt_ps[:])
                            qe_ps = psC.tile([P, w * P], f32, tag="qe4")
                            for k in range(w):
                                nc.tensor.matmul(
                                    out=qe_ps[:, k * P : (k + 1) * P],
                                    lhsT=mt_s[:, k * P : (k + 1) * P],
                                    rhs=Q_all[:, g * P : (g + 1) * P],
                                    start=True,
                                    stop=True,
                                )
                            kq = pc.tile([P, w * P], f32, tag="kq")
                            nc.vector.tensor_tensor(
                                out=kq[:].rearrange("p (t d) -> p t d", d=HID),
                                in0=bk3[:, t0 : t0 + w, :],
                                in1=qe_ps[:].rearrange("p (t d) -> p t d", d=HID)[
                                    :, :w, :
                                ],
                                op=Alu.mult,
                            )
                            s4 = pc.tile([P, w * HEADS], f32, tag="s4")
                            nc.vector.tensor_reduce(
                                out=s4[:],
                                in_=kq[:, : w * P].rearrange(
                                    "p (a x) -> p a x", x=HD
                                ),
                                axis=mybir.AxisListType.X,
                                op=Alu.add,
                            )
                            nc.vector.tensor_scalar(
                                out=s4[:], in0=s4[:],
                                scalar1=10.0, scalar2=-10.0,
                                op0=Alu.min, op1=Alu.max,
                            )
                            rhs4 = pc.tile([P, w * C2], f32, tag="rhs4")
                            r3 = rhs4[:].rearrange("p (t c) -> p t c", c=C2)
                            nc.scalar.activation(
                                out=r3[:, :, HID:],
                                in_=s4[:].rearrange("p (t a) -> p t a", a=HEADS),
                                func=Act.Exp,
                            )
                            nc.vector.tensor_tensor(
                                out=r3[:, :, :HID].rearrange(
                                    "p t (a x) -> p t a x", x=HD
                                ),
                                in0=bv3[:, t0 : t0 + w, :].rearrange(
                                    "p t (a x) -> p t a x", x=HD
                                ),
                                in1=r3[:, :, HID:].to_broadcast([P, w, HEADS, HD]),
                                op=Alu.mult,
                            )
                            for k in range(w):
                                nc.tensor.matmul(
                                    out=wvz_ps[:],
                                    lhsT=m4[:, k * P : (k + 1) * P],
                                    rhs=rhs4[:, k * C2 : (k + 1) * C2],
                                    start=(mm_i == 0),
                                    stop=(mm_i == TT - 1),
                                )
                                mm_i += 1
                    zt = pc.tile([P, HEADS], f32, tag="zt")
                    nc.vector.tensor_scalar_add(zt[:], wvz_ps[:, HID:], 1e-6)
                    rz = pc.tile([P, HEADS], f32, tag="rz")
                    nc.vector.reciprocal(rz[:], zt[:])
                    ob = pc.tile([P, HID], f32, tag="outsb")
                    nc.vector.tensor_tensor(
                        out=ob[:].rearrange("p (a x) -> p a x", x=HD),
                        in0=wvz_ps[:, :HID].rearrange("p (a x) -> p a x", x=HD),
                        in1=rz[:].to_broadcast([P, HEADS, HD]),
                        op=Alu.mult,
                    )
                    nc.sync.dma_start(
                        out=out_e[g * P : (g + 1) * P, :], in_=ob[:]
                    )
    _split_sync_waits(nc)
    return nc


# ----------------------------------------------------------------------------
# Entry point
# ----------------------------------------------------------------------------
def _run(h, W_Q, b_Q, W_K, b_K, W_V, b_V, src, dst, trace=False, tmpdir=None,
         debug=False):
    from concourse.bass_utils import run_bass_kernel_spmd

    h = np.ascontiguousarray(h, np.float32)
    N, HID = h.shape
    HEADS = 8
    E = len(src)

    cfg, per_core, deg_out_pi = _host_prep(N, E, HID, HEADS, src, dst)
    has_bias = bool(
        np.any(np.asarray(b_Q)) or np.any(np.asarray(b_K)) or np.any(np.asarray(b_V))
    )
    nc = _build_program(cfg, has_bias, debug=debug)

    shared = {
        "h": h,
        "wq": np.ascontiguousarray(W_Q, np.float32),
        "wk": np.ascontiguousarray(W_K, np.float32),
        "wv": np.ascontiguousarray(W_V, np.float32),
        "deg_out_pi": deg_out_pi,
    }
    if has_bias:
        shared["bias3"] = np.ascontiguousarray(
            np.stack([b_Q, b_K, b_V]).astype(np.float32)
        )
    in_maps = [dict(shared, **pc) for pc in per_core]
    res = run_bass_kernel_spmd(
        nc, in_maps, list(range(NCORES)), trace=trace, tmpdir=tmpdir
    )
    NPC = cfg["NPC"]
    out = np.concatenate(
        [res.results[c]["out"][:NPC] for c in range(NCORES)], axis=0
    )
    return out, res


def kernel(**inputs):
    out, _ = _run(
        inputs["h"],
        inputs["W_Q"], inputs["b_Q"],
        inputs["W_K"], inputs["b_K"],
        inputs["W_V"], inputs["b_V"],
        np.asarray(inputs["src"]), np.asarray(inputs["dst"]),
    )
    return out.astype(np.float32)


# revision 30
# speedup vs baseline: 3.7915x; 3.3781x over previous
"""Multi-head graph attention layer (GCN-conv QKV + per-edge attention)
on 8 Trainium2 NeuronCores.

Strategy (edge-parallel by dst-owner + node data parallel):
  - Nodes are partitioned across 8 cores (NPC = N/8 per core).  Edges are
    bucketed by the OWNER OF THEIR DST node, so every segment-sum
    (aggregate, wV, z) is core-local: no all-reduce of partials.
  - Within a core, edges are sorted by dst and grouped by 128-node groups.
    A segment-sum over a 128-edge tile is a matmul with a 0/1 selection
    matrix M[e, r] = (dstloc[e] == r) built on-chip with is_equal vs iota.
  - Per-edge gathers of source-node features use gpsimd indirect DMA
    (int32 row indices; one instruction gathers a whole group's edges:
    dest slot [p, t] receives row idx[p, t]).
  - K/V node features are exchanged between cores with one AllGather.

All floating point math happens on-device; the host only does integer
graph preprocessing (bucketing/sorting/padding of edge indices, degree
counting) and the final slice concatenation.
"""

import numpy as np

P = 128
NCORES = 8
SUB = 4  # elementwise op fusion width (tiles per DVE op)


# ----------------------------------------------------------------------------
# Host-side integer preprocessing
# ----------------------------------------------------------------------------
def _host_prep(N, E, HID, HEADS, src, dst):
    NPC = N // NCORES
    assert NPC * NCORES == N, "N must be divisible by 8"
    G = -(-NPC // P)
    NPAD = G * P
    KVR = NCORES * NPAD

    src = np.asarray(src, np.int64)
    dst = np.asarray(dst, np.int64)

    deg_out = np.bincount(src, minlength=N).astype(np.int32)
    deg_in = np.bincount(dst, minlength=N).astype(np.int32)

    kvsrc = (src // NPC) * NPAD + (src % NPC)
    owner = dst // NPC
    r_local = dst % NPC
    g_arr = r_local // P
    rloc = r_local % P

    # counts per (core, group)
    key_full = owner * G + g_arr
    cnt = np.bincount(key_full, minlength=NCORES * G).reshape(NCORES, G)
    T_g = np.maximum(-(-cnt.max(axis=0) // P), 0).astype(np.int64)
    tbase = np.zeros(G + 1, np.int64)
    tbase[1:] = np.cumsum(T_g)
    TILES = int(tbase[-1])

    per_core = []
    for c in range(NCORES):
        sel = np.flatnonzero(owner == c)
        key = g_arr[sel]
        order = np.argsort(key, kind="stable")
        se = sel[order]
        ks = key[order]
        cnts = np.bincount(ks, minlength=G)
        starts = np.concatenate([[0], np.cumsum(cnts)[:-1]])
        within = np.arange(se.size) - np.repeat(starts, cnts)
        slot = tbase[ks] * P + within

        idx32 = np.zeros((P, max(TILES, 1)), np.int32)
        idx32[slot % P, slot // P] = kvsrc[se].astype(np.int32)

        dstloc = np.full((P, max(TILES, 1)), -1.0, np.float32)
        dstloc[slot % P, slot // P] = rloc[se].astype(np.float32)

        # per-core in-degree, partition-inner layout [128, G]
        di = np.ones(NPAD, np.int32)
        di[:NPC] = deg_in[c * NPC : (c + 1) * NPC]
        deg_in_pi = di.reshape(G, P).T.copy()

        per_core.append(
            {
                "idx32": idx32,
                "dstloc": np.ascontiguousarray(dstloc),
                "deg_in_pi": deg_in_pi,
            }
        )

    # replicated out-degree, partition-inner layout [128, NCORES*G]
    do = np.ones(NCORES * NPAD, np.int32)
    for c in range(NCORES):
        do[c * NPAD : c * NPAD + NPC] = deg_out[c * NPC : (c + 1) * NPC]
    deg_out_pi = do.reshape(NCORES * G, P).T.copy()

    cfg = dict(
        N=N, E=E, HID=HID, HEADS=HEADS, HD=HID // HEADS,
        NPC=NPC, G=G, NPAD=NPAD, KVR=KVR,
        T_g=[int(x) for x in T_g],
        tbase=[int(x) for x in tbase], TILES=TILES,
    )
    return cfg, per_core, deg_out_pi


def _split_sync_waits(nc, max_waits=1, nop_waits=1):
    """The walrus build in this container rejects instructions carrying more
    than a couple of semaphore waits (DMA encodings allow just one).  Move
    excess waits onto NoOp instructions inserted just before the offender on
    the same engine."""
    import concourse.mybir as mybir

    n_split = 0
    for bb in nc.main_func.blocks:
        insts = bb.instructions
        i = 0
        while i < len(insts):
            ins = insts[i]
            si = ins.sync_info
            if si is not None and len(si.on_wait) > max_waits:
                waits = list(si.on_wait)
                keep = waits[-max_waits:]
                extra = waits[:-max_waits]
                ins.sync_info = mybir.SyncInfo(
                    on_wait=keep, on_update=list(si.on_update)
                )
                pos = i
                while extra:
                    chunk, extra = extra[:nop_waits], extra[nop_waits:]
                    nop = mybir.InstNoOp(name=f"I-wsplit{n_split}", ins=[], outs=[])
                    n_split += 1
                    nop.engine = ins.engine
                    nop.sync_info = mybir.SyncInfo(on_wait=chunk, on_update=[])
                    insts.insert(pos, nop)
                    pos += 1
                    i += 1
            i += 1
    return n_split


# ----------------------------------------------------------------------------
# Device program
# ----------------------------------------------------------------------------
def _build_program(cfg, has_bias, debug=False):
    import concourse.bass as bass
    import concourse.mybir as mybir
    import concourse.tile as tile
    from concourse.masks import make_identity

    dt = mybir.dt
    f32 = dt.float32
    Alu = mybir.AluOpType
    Act = mybir.ActivationFunctionType

    N, HID, HEADS, HD = cfg["N"], cfg["HID"], cfg["HEADS"], cfg["HD"]
    NPC, G, NPAD, KVR = cfg["NPC"], cfg["G"], cfg["NPAD"], cfg["KVR"]
    T_g, tbase, TILES = cfg["T_g"], cfg["tbase"], cfg["TILES"]
    NC8G = NCORES * G
    FT = NPC // P          # full 128-row tiles per node chunk
    REM = NPC - FT * P     # leftover rows in the last tile
    C2 = HID + HEADS       # wVz psum columns

    nc = bass.Bass()
    h_e = nc.declare_dram_parameter("h_slice", [NPC, HID], f32, isOutput=False)
    wq_e = nc.declare_dram_parameter("wq", [HID, HID], f32, isOutput=False)
    wk_e = nc.declare_dram_parameter("wk", [HID, HID], f32, isOutput=False)
    wv_e = nc.declare_dram_parameter("wv", [HID, HID], f32, isOutput=False)
    dout_e = nc.declare_dram_parameter("deg_out_pi", [P, NC8G], dt.int32, isOutput=False)
    din_e = nc.declare_dram_parameter("deg_in_pi", [P, G], dt.int32, isOutput=False)
    idx_e = nc.declare_dram_parameter("idx32", [P, max(TILES, 1)], dt.int32, isOutput=False)
    dl_e = nc.declare_dram_parameter("dstloc", [P, max(TILES, 1)], f32, isOutput=False)
    if has_bias:
        b_e = nc.declare_dram_parameter("bias3", [3, HID], f32, isOutput=False)
    out_e = nc.declare_dram_parameter("out", [NPAD, HID], f32, isOutput=True)

    bf16 = dt.bfloat16
    hb_d = nc.dram_tensor("h_bounce", [NPC, HID], f32)
    hf_d = nc.dram_tensor("h_full", [NCORES * NPC, HID], f32, addr_space="Shared")
    hw_d = nc.dram_tensor("hw_buf", [KVR, HID], bf16)
    kvlk_d = nc.dram_tensor("kv_local_k", [NPAD, HID], bf16)
    kvlv_d = nc.dram_tensor("kv_local_v", [NPAD, HID], bf16)
    kvfk_d = nc.dram_tensor("kv_full_k", [KVR, HID], bf16, addr_space="Shared")
    kvfv_d = nc.dram_tensor("kv_full_v", [KVR, HID], bf16, addr_space="Shared")
    if debug:
        dbg_hw = nc.declare_dram_parameter("dbg_hw", [KVR, HID], dt.bfloat16, isOutput=True)
        dbg_kvf = nc.declare_dram_parameter(
            "dbg_kvf", [KVR, 2 * HID], dt.bfloat16, isOutput=True
        )
        dbg_q = nc.declare_dram_parameter("dbg_q", [P, NPAD], f32, isOutput=True)
        dbg_agg = nc.declare_dram_parameter("dbg_agg", [G * P, P], f32, isOutput=True)
        T0 = max(T_g[0], 1)
        dbg_buf = nc.declare_dram_parameter("dbg_buf", [P, T0 * HID], f32, isOutput=True)
        dbg_m4 = nc.declare_dram_parameter("dbg_m4", [P, min(SUB, T0) * P], f32, isOutput=True)

    with tile.TileContext(nc) as tc:
        with tc.tile_pool(name="const", bufs=1) as cp:
            # ---- constants & metadata ----
            idn = cp.tile([P, P], f32)
            make_identity(nc, idn[:])
            it_i = cp.tile([P, SUB * P], dt.int32)
            nc.gpsimd.iota(it_i[:], pattern=[[0, SUB], [1, P]], base=0,
                           channel_multiplier=0)
            iota4 = cp.tile([P, SUB * P], f32)
            nc.vector.tensor_copy(iota4[:], it_i[:])

            idx_sb = cp.tile([P, max(TILES, 1)], dt.int32)
            nc.sync.dma_start(out=idx_sb[:], in_=idx_e[:])
            dl_sb = cp.tile([P, max(TILES, 1)], f32)
            nc.sync.dma_start(out=dl_sb[:], in_=dl_e[:])

            wq_sb = cp.tile([P, HID], f32)
            wk_sb = cp.tile([P, HID], f32)
            wv_sb = cp.tile([P, HID], f32)
            nc.sync.dma_start(out=wq_sb[:], in_=wq_e[:])
            nc.sync.dma_start(out=wk_sb[:], in_=wk_e[:])
            nc.sync.dma_start(out=wv_sb[:], in_=wv_e[:])

            # norm_out = rsqrt(max(deg_out, 1)) over all cores' nodes
            no_i = cp.tile([P, NC8G], dt.int32)
            nc.sync.dma_start(out=no_i[:], in_=dout_e[:])
            no_f = cp.tile([P, NC8G], f32)
            nc.vector.tensor_copy(no_f[:], no_i[:])
            nc.vector.tensor_scalar_max(no_f[:], no_f[:], 1.0)
            nc.vector.reciprocal(no_f[:], no_f[:])
            nc.scalar.sqrt(no_f[:], no_f[:])

            ni_i = cp.tile([P, G], dt.int32)
            nc.sync.dma_start(out=ni_i[:], in_=din_e[:])
            ni_f = cp.tile([P, G], f32)
            nc.vector.tensor_copy(ni_f[:], ni_i[:])
            nc.vector.tensor_scalar_max(ni_f[:], ni_f[:], 1.0)
            nc.vector.reciprocal(ni_f[:], ni_f[:])
            nc.scalar.sqrt(ni_f[:], ni_f[:])
            niq = cp.tile([P, G], f32)  # norm_in scaled by 1/sqrt(HD) for Q
            nc.vector.tensor_scalar_mul(niq[:], ni_f[:], 1.0 / float(np.sqrt(HD)))

            if has_bias:
                ones1 = cp.tile([1, P], f32)
                nc.vector.memset(ones1[:], 1.0)
                b_reps = []
                with tc.tile_pool(name="bps", bufs=1, space="PSUM") as bps:
                    for i in range(3):
                        brow = cp.tile([1, HID], f32, tag=f"brow{i}")
                        nc.sync.dma_start(out=brow[:], in_=b_e[i : i + 1, :])
                        ps = bps.tile([P, HID], f32, tag="brep")
                        nc.tensor.matmul(out=ps[:], lhsT=ones1[:],
                                         rhs=brow[:], start=True, stop=True)
                        br = cp.tile([P, HID], f32, tag=f"brep{i}")
                        nc.scalar.copy(br[:], ps[:])
                        b_reps.append(br)

            Q_all = cp.tile([P, NPAD], f32)

            # ---- reassemble full h from per-core slices ----
            nc.sync.dma_start(out=hb_d[:], in_=h_e[:])
            nc.gpsimd.collective_compute(
                "AllGather",
                Alu.bypass,
                replica_groups=[list(range(NCORES))],
                ins=[hb_d[:]],
                outs=[hf_d[:]],
            )

            # ---- prepass: hw = h * norm_out, stored in kv-row numbering ----
            with tc.tile_pool(name="prep", bufs=2) as pp:
                for c in range(NCORES):
                    if FT:
                        hs = pp.tile([P, FT * HID], f32, tag="hslab")
                        hs3 = hs[:].rearrange("p (t d) -> p t d", d=HID)
                        nc.sync.dma_start(
                            out=hs3,
                            in_=hf_d[c * NPC : c * NPC + FT * P, :].rearrange(
                                "(t p) d -> p t d", p=P
                            ),
                        )
                        hb = pp.tile([P, FT * HID], dt.bfloat16, tag="hslab_b")
                        hb3 = hb[:].rearrange("p (t d) -> p t d", d=HID)
                        nc.vector.tensor_tensor(
                            out=hb3,
                            in0=hs3,
                            in1=no_f[:, c * G : c * G + FT].to_broadcast([P, FT, HID]),
                            op=Alu.mult,
                        )
                        nc.sync.dma_start(
                            out=hw_d[c * NPAD : c * NPAD + FT * P, :].rearrange(
                                "(t p) d -> p t d", p=P
                            ),
                            in_=hb3,
                        )
                    if REM:
                        ht = pp.tile([P, HID], f32, tag="hrem")
                        nc.sync.dma_start(
                            out=ht[:REM],
                            in_=hf_d[c * NPC + FT * P : (c + 1) * NPC, :],
                        )
                        hbr = pp.tile([P, HID], dt.bfloat16, tag="hrem_b")
                        nc.vector.tensor_tensor(
                            out=hbr[:REM],
                            in0=ht[:REM],
                            in1=no_f[:REM, c * G + FT : c * G + FT + 1].to_broadcast(
                                [REM, HID]
                            ),
                            op=Alu.mult,
                        )
                        nc.sync.dma_start(
                            out=hw_d[c * NPAD + FT * P : c * NPAD + FT * P + REM, :],
                            in_=hbr[:REM],
                        )

            # ---- phase B: aggregate + QKV projection ----
            with (
                tc.tile_pool(name="pb", bufs=2) as pb,
                tc.tile_pool(name="psB", bufs=2, space="PSUM") as psB,
            ):
                for g in range(G):
                    TT = T_g[g]
                    colbase = tbase[g]
                    aggT_s = pb.tile([P, P], f32, tag="aggTs")
                    if TT == 0:
                        nc.vector.memset(aggT_s[:], 0.0)
                    else:
                        aggT_ps = psB.tile([P, P], f32, tag="aggT")
                        buf = pb.tile([P, TT * HID], dt.bfloat16, tag="gbuf")
                        for t in range(TT):
                            nc.gpsimd.indirect_dma_start(
                                out=buf[:, t * HID : (t + 1) * HID],
                                out_offset=None,
                                in_=hw_d[:, :],
                                in_offset=bass.IndirectOffsetOnAxis(
                                    ap=idx_sb[:, colbase + t : colbase + t + 1],
                                    axis=0,
                                ),
                            )
                        mm_i = 0
                        for t0 in range(0, TT, SUB):
                            w = min(SUB, TT - t0)
                            m4 = pb.tile([P, w * P], dt.bfloat16, tag="m4")
                            nc.vector.tensor_tensor(
                                out=m4[:].rearrange("p (t r) -> p t r", r=P),
                                in0=dl_sb[
                                    :, colbase + t0 : colbase + t0 + w
                                ].to_broadcast([P, w, P]),
                                in1=iota4[:, : w * P].rearrange(
                                    "p (t r) -> p t r", r=P
                                ),
                                op=Alu.is_equal,
                            )
                            if debug and g == 0 and t0 == 0:
                                nc.sync.dma_start(out=dbg_m4[:], in_=m4[:])
                            for k in range(w):
                                t = t0 + k
                                nc.tensor.matmul(
                                    out=aggT_ps[:],
                                    lhsT=buf[:, t * HID : (t + 1) * HID],
                                    rhs=m4[:, k * P : (k + 1) * P],
                                    start=(mm_i == 0),
                                    stop=(mm_i == TT - 1),
                                )
                                mm_i += 1
                        if debug and g == 0:
                            nc.sync.dma_start(out=dbg_buf[:], in_=buf[:])
                        nc.scalar.copy(aggT_s[:], aggT_ps[:])
                    if debug:
                        nc.sync.dma_start(
                            out=dbg_agg[g * P : (g + 1) * P, :], in_=aggT_s[:]
                        )

                    kv_sb = pb.tile([P, 2 * HID], dt.bfloat16, tag="kvsb")
                    plans = (
                        (wq_sb, Q_all[:, g * P : (g + 1) * P], niq, 0),
                        (wk_sb, kv_sb[:, :HID], ni_f, 1),
                        (wv_sb, kv_sb[:, HID:], ni_f, 2),
                    )
                    for w_sb, outap, scol, bi in plans:
                        ps = psB.tile([P, HID], f32, tag="qkv")
                        nc.tensor.matmul(
                            out=ps[:], lhsT=aggT_s[:], rhs=w_sb[:],
                            start=True, stop=True,
                        )
                        if has_bias:
                            tmp = pb.tile([P, HID], f32, tag="btmp")
                            nc.scalar.activation(
                                out=tmp[:], in_=ps[:], func=Act.Copy,
                                scale=ni_f[:, g : g + 1],
                            )
                            nc.vector.tensor_tensor(
                                out=tmp[:], in0=tmp[:], in1=b_reps[bi][:], op=Alu.add
                            )
                            nc.vector.tensor_scalar(
                                out=outap, in0=tmp[:],
                                scalar1=0.0,
                                scalar2=(1.0 / float(np.sqrt(HD)) if bi == 0 else 1.0),
                                op0=Alu.max, op1=Alu.mult,
                            )
                        else:
                            nc.scalar.activation(
                                out=outap, in_=ps[:], func=Act.Relu,
                                scale=scol[:, g : g + 1],
                            )
                    nc.sync.dma_start(
                        out=kvlk_d[g * P : (g + 1) * P, :], in_=kv_sb[:, :HID]
                    )
                    nc.sync.dma_start(
                        out=kvlv_d[g * P : (g + 1) * P, :], in_=kv_sb[:, HID:]
                    )

            # ---- K/V exchange ----
            nc.gpsimd.collective_compute(
                "AllGather",
                Alu.bypass,
                replica_groups=[list(range(NCORES))],
                ins=[kvlk_d[:]],
                outs=[kvfk_d[:]],
            )
            nc.gpsimd.collective_compute(
                "AllGather",
                Alu.bypass,
                replica_groups=[list(range(NCORES))],
                ins=[kvlv_d[:]],
                outs=[kvfv_d[:]],
            )
            if debug:
                nc.sync.dma_start(out=dbg_hw[:], in_=hw_d[:])
                nc.sync.dma_start(out=dbg_kvf[:, :HID], in_=kvfk_d[:])
                nc.sync.dma_start(out=dbg_kvf[:, HID:], in_=kvfv_d[:])
                nc.sync.dma_start(out=dbg_q[:], in_=Q_all[:])

            # ---- phase C: per-edge attention ----
            with (
                tc.tile_pool(name="pc", bufs=2) as pc,
                tc.tile_pool(name="psC", bufs=2, space="PSUM") as psC,
            ):
                for g in range(G):
                    TT = T_g[g]
                    if TT == 0:
                        ob = pc.tile([P, HID], f32, tag="outsb")
                        nc.vector.memset(ob[:], 0.0)
                        nc.sync.dma_start(
                            out=out_e[g * P : (g + 1) * P, :], in_=ob[:]
                        )
                        continue
                    wvz_ps = psC.tile([P, C2], f32, tag="wvz")
                    mm_i = 0
                    for T, colbase in ((TT, tbase[g]),):
                        bufk = pc.tile([P, T * HID], dt.bfloat16, tag="kbuf")
                        bufv = pc.tile([P, T * HID], dt.bfloat16, tag="vbuf")
                        bk3 = bufk[:].rearrange("p (t x) -> p t x", x=HID)
                        bv3 = bufv[:].rearrange("p (t x) -> p t x", x=HID)
                        for t in range(T):
                            nc.gpsimd.indirect_dma_start(
                                out=bufk[:, t * HID : (t + 1) * HID],
                                out_offset=None,
                                in_=kvfk_d[:, :],
                                in_offset=bass.IndirectOffsetOnAxis(
                                    ap=idx_sb[:, colbase + t : colbase + t + 1],
                                    axis=0,
                                ),
                            )
                            nc.gpsimd.indirect_dma_start(
                                out=bufv[:, t * HID : (t + 1) * HID],
                                out_offset=None,
                                in_=kvfv_d[:, :],
                                in_offset=bass.IndirectOffsetOnAxis(
                                    ap=idx_sb[:, colbase + t : colbase + t + 1],
                                    axis=0,
                                ),
                            )
                        for t0 in range(0, T, SUB):
                            w = min(SUB, T - t0)
                            m4 = pc.tile([P, w * P], f32, tag="m4c")
                            nc.vector.tensor_tensor(
                                out=m4[:].rearrange("p (t r) -> p t r", r=P),
                                in0=dl_sb[
                                    :, colbase + t0 : colbase + t0 + w
                                ].to_broadcast([P, w, P]),
                                in1=iota4[:, : w * P].rearrange(
                                    "p (t r) -> p t r", r=P
                                ),
                                op=Alu.is_equal,
                            )
                            mt_ps = psC.tile([P, w * P], f32, tag="mt4")
                            for k in range(w):
                                nc.tensor.transpose(
                                    out=mt_ps[:, k * P : (k + 1) * P],
                                    in_=m4[:, k * P : (k + 1) * P],
                                    identity=idn[:],
                                )
                            mt_s = pc.tile([P, w * P], f32, tag="mt4s")
                            nc.scalar.copy(mt_s[:], mt_ps[:])
                            qe_ps = psC.tile([P, w * P], f32, tag="qe4")
                            for k in range(w):
                                nc.tensor.matmul(
                                    out=qe_ps[:, k * P : (k + 1) * P],
                                    lhsT=mt_s[:, k * P : (k + 1) * P],
                                    rhs=Q_all[:, g * P : (g + 1) * P],
                                    start=True,
                                    stop=True,
                                )
                            kq = pc.tile([P, w * P], f32, tag="kq")
                            nc.vector.tensor_tensor(
                                out=kq[:].rearrange("p (t d) -> p t d", d=HID),
                                in0=bk3[:, t0 : t0 + w, :],
                                in1=qe_ps[:].rearrange("p (t d) -> p t d", d=HID)[
                                    :, :w, :
                                ],
                                op=Alu.mult,
                            )
                            s4 = pc.tile([P, w * HEADS], f32, tag="s4")
                            nc.vector.tensor_reduce(
                                out=s4[:],
                                in_=kq[:, : w * P].rearrange(
                                    "p (a x) -> p a x", x=HD
                                ),
                                axis=mybir.AxisListType.X,
                                op=Alu.add,
                            )
                            nc.vector.tensor_scalar(
                                out=s4[:], in0=s4[:],
                                scalar1=10.0, scalar2=-10.0,
                                op0=Alu.min, op1=Alu.max,
                            )
                            rhs4 = pc.tile([P, w * C2], f32, tag="rhs4")
                            r3 = rhs4[:].rearrange("p (t c) -> p t c", c=C2)
                            nc.scalar.activation(
                                out=r3[:, :, HID:],
                                in_=s4[:].rearrange("p (t a) -> p t a", a=HEADS),
                                func=Act.Exp,
                            )
                            nc.vector.tensor_tensor(
                                out=r3[:, :, :HID].rearrange(
                                    "p t (a x) -> p t a x", x=HD
                                ),
                                in0=bv3[:, t0 : t0 + w, :].rearrange(
                                    "p t (a x) -> p t a x", x=HD
                                ),
                                in1=r3[:, :, HID:].to_broadcast([P, w, HEADS, HD]),
                                op=Alu.mult,
                            )
                            for k in range(w):
                                nc.tensor.matmul(
                                    out=wvz_ps[:],
                                    lhsT=m4[:, k * P : (k + 1) * P],
                                    rhs=rhs4[:, k * C2 : (k + 1) * C2],
                                    start=(mm_i == 0),
                                    stop=(mm_i == TT - 1),
                                )
                                mm_i += 1
                    zt = pc.tile([P, HEADS], f32, tag="zt")
                    nc.vector.tensor_scalar_add(zt[:], wvz_ps[:, HID:], 1e-6)
                    rz = pc.tile([P, HEADS], f32, tag="rz")
                    nc.vector.reciprocal(rz[:], zt[:])
                    ob = pc.tile([P, HID], f32, tag="outsb")
                    nc.vector.tensor_tensor(
                        out=ob[:].rearrange("p (a x) -> p a x", x=HD),
                        in0=wvz_ps[:, :HID].rearrange("p (a x) -> p a x", x=HD),
                        in1=rz[:].to_broadcast([P, HEADS, HD]),
                        op=Alu.mult,
                    )
                    nc.sync.dma_start(
                        out=out_e[g * P : (g + 1) * P, :], in_=ob[:]
                    )
    _split_sync_waits(nc)
    return nc


# ----------------------------------------------------------------------------
# Entry point
# ----------------------------------------------------------------------------
def _run(h, W_Q, b_Q, W_K, b_K, W_V, b_V, src, dst, trace=False, tmpdir=None,
         debug=False):
    from concourse.bass_utils import run_bass_kernel_spmd

    h = np.ascontiguousarray(h, np.float32)
    N, HID = h.shape
    HEADS = 8
    E = len(src)

    cfg, per_core, deg_out_pi = _host_prep(N, E, HID, HEADS, src, dst)
    has_bias = bool(
        np.any(np.asarray(b_Q)) or np.any(np.asarray(b_K)) or np.any(np.asarray(b_V))
    )
    nc = _build_program(cfg, has_bias, debug=debug)

    NPCh = cfg["NPC"]
    for c in range(NCORES):
        per_core[c]["h_slice"] = np.ascontiguousarray(h[c * NPCh : (c + 1) * NPCh])
    shared = {
        "wq": np.ascontiguousarray(W_Q, np.float32),
        "wk": np.ascontiguousarray(W_K, np.float32),
        "wv": np.ascontiguousarray(W_V, np.float32),
        "deg_out_pi": deg_out_pi,
    }
    if has_bias:
        shared["bias3"] = np.ascontiguousarray(
            np.stack([b_Q, b_K, b_V]).astype(np.float32)
        )
    in_maps = [dict(shared, **pc) for pc in per_core]
    res = run_bass_kernel_spmd(
        nc, in_maps, list(range(NCORES)), trace=trace, tmpdir=tmpdir
    )
    NPC = cfg["NPC"]
    out = np.concatenate(
        [res.results[c]["out"][:NPC] for c in range(NCORES)], axis=0
    )
    return out, res


def kernel(**inputs):
    out, _ = _run(
        inputs["h"],
        inputs["W_Q"], inputs["b_Q"],
        inputs["W_K"], inputs["b_K"],
        inputs["W_V"], inputs["b_V"],
        np.asarray(inputs["src"]), np.asarray(inputs["dst"]),
    )
    return out.astype(np.float32)
